# revision 8
# baseline (speedup 1.0000x reference)
"""Trainium2 Bass kernel for nn_BottomLevelDecoderRNN.

2-layer GRU decoder, H=1024, over S=16 steps for E*B = 16*128 = 2048
independent sequences. Data-parallel over 8 NeuronCores: each core owns
2 conductor embeddings (256 rows).

Dataflow (per core, everything transposed as [feature, row]):
  init:    h-init: t0T = tanh(fc_init_w @ cflatT + b)  -> h1T, h2T [H, 256]
           cached = Wc @ cflatT + bih1                 (Wc = g1_wih[:, :C])
           gi1[s] = Wp @ prevT[s] + cached  (all s, batched N=512) -> DRAM
  step s:  g1T = whh1 @ h1T  (+ gi1[s] streamed back)  -> GRU1 gates -> h1T'
           g2T = wih2 @ h1T' + whh2 @ h2T              -> GRU2 gates -> h2T'
           yT  = fco_w @ h2T' + fco_b                  -> out
Matmuls run with fp16 operands (weights pre-transposed on host into PE
stationary-tile layout), fp32 PSUM accumulation. whh2 streams from HBM
per step (SBUF cannot hold all three 3072x1024 matrices + state).
"""
import numpy as np

E, B, C, H, D = 16, 128, 512, 1024, 130
S = 16
NCORES = 8
EPC = E // NCORES        # 2 embeddings per core
R = EPC * B              # 256 rows per core
KH = H // 128            # 8 h k-tiles
MG = 3 * H // 128        # 24 gate m-tiles
MRZ = 2 * H // 128       # 16 rz m-tiles
NJ = H // 128            # 8 n/h tiles
KC = C // 128            # 4 c k-tiles
MI = 2 * H // 128        # 16 init m-tiles
NCH = S // 2             # 8 init gi1 chunks (2 steps each, N=512)

# bias tile column layout ([128, NBIAS] fp32)
B_INIT = 0      # 16: fc_init_b
B_RZ1 = 16      # 16: bhh1[:2H]
B_N1H = 32      # 8:  bhh1[2H:]
B_IH1 = 40      # 24: bih1
B_RZ2 = 64      # 16: bih2[:2H]+bhh2[:2H]
B_N2H = 80      # 8:  bhh2[2H:]
B_N2I = 88      # 8:  bih2[2H:]
B_FCO = 96      # 2:  fco_b
NBIAS = 98

_cache = {}


def _wtiles(w_t, nm, nk):
    """[Kfull, Mfull] (already transposed: w_t = W.T) -> [nm, 128, nk*128]
    stationary-tile chunks: chunk[m][p, k*128+c] = w_t[128k+p, 128m+c]."""
    Kf, Mf = w_t.shape
    assert Kf == nk * 128 and Mf == nm * 128
    return np.ascontiguousarray(
        w_t.reshape(nk, 128, nm, 128).transpose(2, 1, 0, 3).reshape(nm, 128, nk * 128)
    ).astype(np.float16)


def _bias_cols(vec, n):
    """[n*128] -> [128, n] (col j = vec[128j:128j+128])"""
    return np.ascontiguousarray(vec.reshape(n, 128).T).astype(np.float32)


def build_program():
    import concourse.tile as tile
    from concourse import bacc, mybir

    f32, f16 = mybir.dt.float32, mybir.dt.float16
    Sig = mybir.ActivationFunctionType.Sigmoid
    Tanh = mybir.ActivationFunctionType.Tanh
    Ident = mybir.ActivationFunctionType.Identity

    nc = bacc.Bacc("TRN2", target_bir_lowering=False, debug=False,
                   enable_asserts=False, num_devices=NCORES)

    def din(name, shape, dt=f16):
        return nc.dram_tensor(name, shape, dt, kind="ExternalInput").ap()

    cflatT = din("cflatT", [KC, 128, R])
    prevT0 = din("prevT0", [128, S * R])     # columns ordered (s, r)
    prevT1 = din("prevT1", [2, S * R])
    w1h = din("w1h", [MG, 128, KH * 128])
    w2i = din("w2i", [MG, 128, KH * 128])
    w2h = din("w2h", [MG, 128, KH * 128])
    wp0 = din("wp0", [128, MG * 128])
    wp1 = din("wp1", [2, MG * 128])
    wc = din("wc", [MG, 128, KC * 128])
    wini = din("wini", [MI, 128, KC * 128])
    wfco = din("wfco", [128, KH * D])
    biases = din("biases", [128, NBIAS], f32)
    yT = nc.dram_tensor("yT", [S, 132, R], f32, kind="ExternalOutput").ap()
    gi1d = nc.dram_tensor("gi1d", [S, 128, MG * R], f16).ap()  # internal

    with tile.TileContext(nc) as tc:
        with tc.tile_pool(name="const", bufs=1) as const, \
             tc.tile_pool(name="stream", bufs=8) as stream, \
             tc.tile_pool(name="gi1p", bufs=2) as gi1p, \
             tc.tile_pool(name="state", bufs=2) as state, \
             tc.tile_pool(name="gates", bufs=1) as gates, \
             tc.tile_pool(name="tmp", bufs=4) as tmp, \
             tc.tile_pool(name="giwp", bufs=6) as giwp, \
             tc.tile_pool(name="ghb2p", bufs=9) as ghb2p, \
             tc.tile_pool(name="prevp", bufs=3) as prevp, \
             tc.tile_pool(name="outp", bufs=2) as outp, \
             tc.tile_pool(name="psA", bufs=4, space="PSUM") as psA, \
             tc.tile_pool(name="psB", bufs=2, space="PSUM") as psB, \
             tc.tile_pool(name="psC", bufs=2, space="PSUM") as psC:

            # ---- constant loads ----
            bias_sb = const.tile([128, NBIAS], f32, tag="bias")
            nc.sync.dma_start(bias_sb[:], biases[:])
            cfl_sb = const.tile([128, KC * R], f16, tag="cfl")
            for k in range(KC):
                nc.sync.dma_start(cfl_sb[:, k * R:(k + 1) * R], cflatT[k])
            wp0_sb = const.tile([128, MG * 128], f16, tag="wp0")
            nc.sync.dma_start(wp0_sb[:], wp0[:])
            wp1_sb = const.tile([2, MG * 128], f16, tag="wp1")
            nc.sync.dma_start(wp1_sb[:], wp1[:])
            wfco_sb = const.tile([128, KH * D], f16, tag="wfco")
            nc.sync.dma_start(wfco_sb[:], wfco[:])

            def bias_ap(col):
                return bias_sb[:, col:col + 1]

            # ---- h init: t0T = tanh(wini @ cflatT + binit) ----
            h1T = state.tile([128, KH * R], f16, tag="h1")
            h2T = state.tile([128, KH * R], f16, tag="h2")
            for m in range(MI):
                wchunk = stream.tile([128, KC * 128], f16, tag="stream")
                nc.sync.dma_start(wchunk[:], wini[m])
                ps = psA.tile([128, R], f32, tag="rz")
                for k in range(KC):
                    nc.tensor.matmul(ps[:], wchunk[:, k * 128:(k + 1) * 128],
                                     cfl_sb[:, k * R:(k + 1) * R],
                                     start=(k == 0), stop=(k == KC - 1))
                dst = h1T if m < NJ else h2T
                j = m % NJ
                nc.scalar.activation(dst[:, j * R:(j + 1) * R], ps[:], Tanh,
                                     bias=bias_ap(B_INIT + m))

            # ---- cached = Wc @ cflatT + bih1 ----
            # lives in the gi1 streaming pool: dead once the step loop starts
            cached_sb = gi1p.tile([128, MG * R], f16, tag="gi1")
            for m in range(MG):
                wchunk = stream.tile([128, KC * 128], f16, tag="stream")
                nc.sync.dma_start(wchunk[:], wc[m])
                ps = psA.tile([128, R], f32, tag="rz")
                for k in range(KC):
                    nc.tensor.matmul(ps[:], wchunk[:, k * 128:(k + 1) * 128],
                                     cfl_sb[:, k * R:(k + 1) * R],
                                     start=(k == 0), stop=(k == KC - 1))
                nc.scalar.activation(cached_sb[:, m * R:(m + 1) * R], ps[:], Ident,
                                     bias=bias_ap(B_IH1 + m))

            # ---- gi1[s] = Wp @ prevT + cached, batched over (s, r) ----
            for ch in range(NCH):
                pv0c = stream.tile([128, 2 * R], f16, tag="stream")
                nc.sync.dma_start(pv0c[:], prevT0[:, ch * 2 * R:(ch + 1) * 2 * R])
                pv1c = prevp.tile([2, 2 * R], f16, tag="pv1")
                nc.sync.dma_start(pv1c[:], prevT1[:, ch * 2 * R:(ch + 1) * 2 * R])
                for m in range(MG):
                    ps = psA.tile([128, 2 * R], f32, tag="rz")
                    nc.tensor.matmul(ps[:], wp0_sb[:, m * 128:(m + 1) * 128],
                                     pv0c[:], start=True, stop=False)
                    nc.tensor.matmul(ps[:], wp1_sb[:, m * 128:(m + 1) * 128],
                                     pv1c[:], start=False, stop=True)
                    for half in range(2):
                        giw = giwp.tile([128, R], f16, tag="giw")
                        nc.vector.tensor_add(giw[:], ps[:, half * R:(half + 1) * R],
                                             cached_sb[:, m * R:(m + 1) * R])
                        nc.gpsimd.dma_start(
                            gi1d[ch * 2 + half, :, m * R:(m + 1) * R], giw[:])

            # ---- resident big weights ----
            w1h_sb = const.tile([128, MG * KH * 128], f16, tag="w1h")
            for m in range(MG):
                nc.sync.dma_start(w1h_sb[:, m * 1024:(m + 1) * 1024], w1h[m])
            w2i_sb = const.tile([128, MG * KH * 128], f16, tag="w2i")
            for m in range(MG):
                nc.sync.dma_start(w2i_sb[:, m * 1024:(m + 1) * 1024], w2i[m])

            def w1h_t(m, k):
                return w1h_sb[:, m * 1024 + k * 128: m * 1024 + (k + 1) * 128]

            def w2i_t(m, k):
                return w2i_sb[:, m * 1024 + k * 128: m * 1024 + (k + 1) * 128]

            def hslice(t, j):
                return t[:, j * R:(j + 1) * R]

            def fco_step(h2T_cur, s):
                for mo, msz, bc in [(0, 128, B_FCO), (128, 2, B_FCO + 1)]:
                    ps = psA.tile([128, R], f32, tag="rz")
                    for k in range(KH):
                        nc.tensor.matmul(ps[0:msz, :],
                                         wfco_sb[:, k * D + mo: k * D + mo + msz],
                                         hslice(h2T_cur, k),
                                         start=(k == 0), stop=(k == KH - 1))
                    ysb = outp.tile([128, R], f32, tag="y")
                    nc.scalar.activation(ysb[0:msz, :], ps[0:msz, :], Ident,
                                         bias=bias_sb[0:msz, bc:bc + 1])
                    nc.sync.dma_start(yT[s, mo:mo + msz, :], ysb[0:msz, :])

            h2T_done = []  # (h2T tile, step) pending fco

            for s in range(S):
                gi1_t = gi1p.tile([128, MG * R], f16, tag="gi1")
                nc.sync.dma_start(gi1_t[:], gi1d[s])

                # ---------- GRU1 ----------
                r1 = gates.tile([128, NJ * R], f16, tag="rg")
                z1 = gates.tile([128, NJ * R], f16, tag="zg")
                n1 = gates.tile([128, NJ * R], f16, tag="ng")
                for m in range(MRZ):
                    ps = psA.tile([128, R], f32, tag="rz")
                    for k in range(KH):
                        nc.tensor.matmul(ps[:], w1h_t(m, k), hslice(h1T, k),
                                         start=(k == 0), stop=(k == KH - 1))
                    nc.vector.tensor_add(ps[:], ps[:], gi1_t[:, m * R:(m + 1) * R])
                    dst = r1 if m < NJ else z1
                    nc.scalar.activation(hslice(dst, m % NJ), ps[:], Sig,
                                         bias=bias_ap(B_RZ1 + m))
                h1T_new = state.tile([128, KH * R], f16, tag="h1")
                for j in range(NJ):
                    m = MRZ + j
                    psh = psC.tile([128, R], f32, tag="ghn")
                    for k in range(KH):
                        nc.tensor.matmul(psh[:], w1h_t(m, k), hslice(h1T, k),
                                         start=(k == 0), stop=(k == KH - 1))
                    ghb = tmp.tile([128, R], f16, tag="ghb")
                    nc.scalar.activation(ghb[:], psh[:], Ident, bias=bias_ap(B_N1H + j))
                    tt = tmp.tile([128, R], f16, tag="tt")
                    nc.vector.tensor_mul(tt[:], hslice(r1, j), ghb[:])
                    narg = tmp.tile([128, R], f16, tag="narg")
                    nc.vector.tensor_add(narg[:], tt[:], gi1_t[:, m * R:(m + 1) * R])
                    nc.scalar.activation(hslice(n1, j), narg[:], Tanh, bias=0.0)
                    d = tmp.tile([128, R], f16, tag="d")
                    nc.vector.tensor_sub(d[:], hslice(h1T, j), hslice(n1, j))
                    nc.vector.tensor_mul(d[:], hslice(z1, j), d[:])
                    nc.vector.tensor_add(hslice(h1T_new, j), hslice(n1, j), d[:])

                # fco for previous step (PE filler while GRU1 gates finish)
                if h2T_done:
                    fco_step(*h2T_done.pop())

                # ---------- GRU2 ----------
                r2 = gates.tile([128, NJ * R], f16, tag="rg")
                z2 = gates.tile([128, NJ * R], f16, tag="zg")
                n2 = gates.tile([128, NJ * R], f16, tag="ng")
                # n-gate gh part first: independent of h1T_new
                ghb2s = []
                for j in range(NJ):
                    m = MRZ + j
                    wch = stream.tile([128, KH * 128], f16, tag="stream")
                    nc.sync.dma_start(wch[:], w2h[m])
                    psh = psC.tile([128, R], f32, tag="ghn")
                    for k in range(KH):
                        nc.tensor.matmul(psh[:], wch[:, k * 128:(k + 1) * 128],
                                         hslice(h2T, k),
                                         start=(k == 0), stop=(k == KH - 1))
                    ghb2 = ghb2p.tile([128, R], f16, tag="ghb2")
                    nc.scalar.activation(ghb2[:], psh[:], Ident, bias=bias_ap(B_N2H + j))
                    ghb2s.append(ghb2)
                for m in range(MRZ):
                    wch = stream.tile([128, KH * 128], f16, tag="stream")
                    nc.sync.dma_start(wch[:], w2h[m])
                    ps = psA.tile([128, R], f32, tag="rz")
                    for k in range(KH):
                        nc.tensor.matmul(ps[:], wch[:, k * 128:(k + 1) * 128],
                                         hslice(h2T, k),
                                         start=(k == 0), stop=False)
                    for k in range(KH):
                        nc.tensor.matmul(ps[:], w2i_t(m, k), hslice(h1T_new, k),
                                         start=False, stop=(k == KH - 1))
                    dst = r2 if m < NJ else z2
                    nc.scalar.activation(hslice(dst, m % NJ), ps[:], Sig,
                                         bias=bias_ap(B_RZ2 + m))
                h2T_new = state.tile([128, KH * R], f16, tag="h2")
                for j in range(NJ):
                    m = MRZ + j
                    psg = psB.tile([128, R], f32, tag="gin")
                    for k in range(KH):
                        nc.tensor.matmul(psg[:], w2i_t(m, k), hslice(h1T_new, k),
                                         start=(k == 0), stop=(k == KH - 1))
                    tt = tmp.tile([128, R], f16, tag="tt")
                    nc.vector.tensor_mul(tt[:], hslice(r2, j), ghb2s[j][:])
                    nc.vector.tensor_add(psg[:], psg[:], tt[:])
                    nc.scalar.activation(hslice(n2, j), psg[:], Tanh,
                                         bias=bias_ap(B_N2I + j))
                    d = tmp.tile([128, R], f16, tag="d")
                    nc.vector.tensor_sub(d[:], hslice(h2T, j), hslice(n2, j))
                    nc.vector.tensor_mul(d[:], hslice(z2, j), d[:])
                    nc.vector.tensor_add(hslice(h2T_new, j), hslice(n2, j), d[:])

                h1T, h2T = h1T_new, h2T_new
                h2T_done.append((h2T, s))

            fco_step(*h2T_done.pop())

    nc.compile()
    return nc


def prep_inputs(c, target, fc_init_w, fc_init_b, g1_wih, g1_whh, g1_bih, g1_bhh,
                g2_wih, g2_whh, g2_bih, g2_bhh, fco_w, fco_b):
    """Host-side shard/layout prep. Returns per-core input maps."""
    f16 = np.float16
    T = E * S
    c = np.asarray(c, np.float32)
    target = np.asarray(target, np.float32)

    # weights: stationary-tile layouts (shared across cores)
    w1h_a = _wtiles(np.asarray(g1_whh, np.float32).T, MG, KH)
    w2i_a = _wtiles(np.asarray(g2_wih, np.float32).T, MG, KH)
    w2h_a = _wtiles(np.asarray(g2_whh, np.float32).T, MG, KH)
    wc_a = _wtiles(np.asarray(g1_wih, np.float32)[:, :C].T, MG, KC)
    wini_a = _wtiles(np.asarray(fc_init_w, np.float32).T, MI, KC)
    wp_t = np.asarray(g1_wih, np.float32)[:, C:].T      # [130, 3072]
    wp0_a = np.ascontiguousarray(wp_t[:128]).astype(f16)   # [128, 24*128]
    wp1_a = np.ascontiguousarray(wp_t[128:]).astype(f16)   # [2, 24*128]
    wfco_a = np.ascontiguousarray(
        np.asarray(fco_w, np.float32).T.reshape(KH, 128, D)
        .transpose(1, 0, 2).reshape(128, KH * D)).astype(f16)

    bias = np.zeros((128, NBIAS), np.float32)
    bias[:, B_INIT:B_INIT + MI] = _bias_cols(np.asarray(fc_init_b, np.float32), MI)
    bhh1 = np.asarray(g1_bhh, np.float32)
    bih1 = np.asarray(g1_bih, np.float32)
    bhh2 = np.asarray(g2_bhh, np.float32)
    bih2 = np.asarray(g2_bih, np.float32)
    bias[:, B_RZ1:B_RZ1 + 16] = _bias_cols(bhh1[:2 * H], 16)
    bias[:, B_N1H:B_N1H + 8] = _bias_cols(bhh1[2 * H:], 8)
    bias[:, B_IH1:B_IH1 + 24] = _bias_cols(bih1, 24)
    bias[:, B_RZ2:B_RZ2 + 16] = _bias_cols(bih2[:2 * H] + bhh2[:2 * H], 16)
    bias[:, B_N2H:B_N2H + 8] = _bias_cols(bhh2[2 * H:], 8)
    bias[:, B_N2I:B_N2I + 8] = _bias_cols(bih2[2 * H:], 8)
    fco_b = np.asarray(fco_b, np.float32)
    bias[:, B_FCO] = fco_b[:128]
    bias[0:2, B_FCO + 1] = fco_b[128:130]

    prev_full = np.concatenate(
        [np.zeros((B, 1, D), np.float32), target[:, :T - 1]], axis=1)  # [B,T,D]

    in_maps = []
    for core in range(NCORES):
        e0 = core * EPC
        cf = c[e0:e0 + EPC].reshape(R, C)                  # [256, 512]
        cfT = np.ascontiguousarray(cf.T.reshape(KC, 128, R)).astype(f16)
        pv = prev_full[:, e0 * S:(e0 + EPC) * S]           # [B, 32, D]
        pv = pv.reshape(B, EPC, S, D).transpose(2, 1, 0, 3).reshape(S, R, D)
        pvT = np.ascontiguousarray(pv.transpose(2, 0, 1).reshape(D, S * R))  # [D,(s,r)]
        in_maps.append({
            "cflatT": cfT,
            "prevT0": np.ascontiguousarray(pvT[:128]).astype(f16),
            "prevT1": np.ascontiguousarray(pvT[128:130]).astype(f16),
            "w1h": w1h_a, "w2i": w2i_a, "w2h": w2h_a,
            "wp0": wp0_a, "wp1": wp1_a, "wc": wc_a, "wini": wini_a,
            "wfco": wfco_a, "biases": bias,
        })
    return in_maps


def assemble_output(results):
    """Per-core yT [S, 132, R] f32 -> full [B, T, D] f32."""
    T = E * S
    out = np.empty((B, T, D), np.float32)
    for core in range(NCORES):
        yt = results[core]["yT"]            # [S, 132, R]
        for ei in range(EPC):
            e = core * EPC + ei
            blk = yt[:, :D, ei * 128:(ei + 1) * 128]   # [S, D, 128]
            out[:, e * S:(e + 1) * S, :] = blk.transpose(2, 0, 1)
    return out


def kernel(c, target, length, batch_size, fc_init_w, fc_init_b,
           g1_wih, g1_whh, g1_bih, g1_bhh,
           g2_wih, g2_whh, g2_bih, g2_bhh, fco_w, fco_b):
    from concourse.bass_utils import run_bass_kernel_spmd

    if "nc" not in _cache:
        _cache["nc"] = build_program()
    nc = _cache["nc"]
    in_maps = prep_inputs(c, target, fc_init_w, fc_init_b,
                          g1_wih, g1_whh, g1_bih, g1_bhh,
                          g2_wih, g2_whh, g2_bih, g2_bhh, fco_w, fco_b)
    res = run_bass_kernel_spmd(nc, in_maps, list(range(NCORES)))
    return assemble_output(res.results)


# revision 11
# speedup vs baseline: 1.0242x; 1.0242x over previous
"""Trainium2 Bass kernel for nn_BottomLevelDecoderRNN.

2-layer GRU decoder, H=1024, over S=16 steps for E*B = 16*128 = 2048
independent sequences. Data-parallel over 8 NeuronCores: each core owns
2 conductor embeddings (256 rows).

Dataflow (per core, everything transposed as [feature, row]):
  init:    h-init: t0T = tanh(fc_init_w @ cflatT + b)  -> h1T, h2T [H, 256]
           cached = Wc @ cflatT + bih1                 (Wc = g1_wih[:, :C])
           gi1[s] = Wp @ prevT[s] + cached  (all s, batched N=512) -> DRAM
  step s:  g1T = whh1 @ h1T  (+ gi1[s] streamed back)  -> GRU1 gates -> h1T'
           g2T = wih2 @ h1T' + whh2 @ h2T              -> GRU2 gates -> h2T'
           yT  = fco_w @ h2T' + fco_b                  -> out
Matmuls run with fp16 operands (weights pre-transposed on host into PE
stationary-tile layout), fp32 PSUM accumulation. whh2 streams from HBM
per step (SBUF cannot hold all three 3072x1024 matrices + state).
"""
import numpy as np

E, B, C, H, D = 16, 128, 512, 1024, 130
S = 16
NCORES = 8
EPC = E // NCORES        # 2 embeddings per core
R = EPC * B              # 256 rows per core
KH = H // 128            # 8 h k-tiles
MG = 3 * H // 128        # 24 gate m-tiles
MRZ = 2 * H // 128       # 16 rz m-tiles
NJ = H // 128            # 8 n/h tiles
KC = C // 128            # 4 c k-tiles
MI = 2 * H // 128        # 16 init m-tiles
NCH = S // 2             # 8 init gi1 chunks (2 steps each, N=512)

# bias tile column layout ([128, NBIAS] fp32)
B_INIT = 0      # 16: fc_init_b
B_RZ1 = 16      # 16: bhh1[:2H]
B_N1H = 32      # 8:  bhh1[2H:]
B_IH1 = 40      # 24: bih1
B_RZ2 = 64      # 16: bih2[:2H]+bhh2[:2H]
B_N2H = 80      # 8:  bhh2[2H:]
B_N2I = 88      # 8:  bih2[2H:]
B_FCO = 96      # 2:  fco_b
NBIAS = 98

_cache = {}


def _wtiles(w_t, nm, nk):
    """[Kfull, Mfull] (already transposed: w_t = W.T) -> [nm, 128, nk*128]
    stationary-tile chunks: chunk[m][p, k*128+c] = w_t[128k+p, 128m+c]."""
    Kf, Mf = w_t.shape
    assert Kf == nk * 128 and Mf == nm * 128
    return np.ascontiguousarray(
        w_t.reshape(nk, 128, nm, 128).transpose(2, 1, 0, 3).reshape(nm, 128, nk * 128)
    ).astype(np.float16)


def _bias_cols(vec, n):
    """[n*128] -> [128, n] (col j = vec[128j:128j+128])"""
    return np.ascontiguousarray(vec.reshape(n, 128).T).astype(np.float32)


def build_program():
    import concourse.tile as tile
    from concourse import bacc, mybir

    f32, f16 = mybir.dt.float32, mybir.dt.float16
    Sig = mybir.ActivationFunctionType.Sigmoid
    Tanh = mybir.ActivationFunctionType.Tanh
    Ident = mybir.ActivationFunctionType.Identity

    nc = bacc.Bacc("TRN2", target_bir_lowering=False, debug=False,
                   enable_asserts=False, num_devices=NCORES)

    def din(name, shape, dt=f16):
        return nc.dram_tensor(name, shape, dt, kind="ExternalInput").ap()

    cflatT = din("cflatT", [KC, 128, R])
    prevT0 = din("prevT0", [128, S * R])     # columns ordered (s, r)
    prevT1 = din("prevT1", [2, S * R])
    w1h = din("w1h", [MG, 128, KH * 128])
    w2i = din("w2i", [MG, 128, KH * 128])
    w2h = din("w2h", [MG, 128, KH * 128])
    wp0 = din("wp0", [128, MG * 128])
    wp1 = din("wp1", [2, MG * 128])
    wc = din("wc", [MG, 128, KC * 128])
    wini = din("wini", [MI, 128, KC * 128])
    wfco = din("wfco", [128, KH * D])
    biases = din("biases", [128, NBIAS], f32)
    yT = nc.dram_tensor("yT", [S, 132, R], f32, kind="ExternalOutput").ap()
    # internal scratch: per chunk ch (= steps 2ch, 2ch+1), m-block layout
    # [128, m*512 + (half*256 + r)]
    gi1d = nc.dram_tensor("gi1d", [NCH, 128, MG * 2 * R], f16).ap()

    with tile.TileContext(nc) as tc:
        with tc.tile_pool(name="const", bufs=1) as const, \
             tc.tile_pool(name="stream", bufs=8) as stream, \
             tc.tile_pool(name="gi1p", bufs=2) as gi1p, \
             tc.tile_pool(name="state", bufs=2) as state, \
             tc.tile_pool(name="gates", bufs=1) as gates, \
             tc.tile_pool(name="tmp", bufs=4) as tmp, \
             tc.tile_pool(name="giwp", bufs=6) as giwp, \
             tc.tile_pool(name="ghb2p", bufs=9) as ghb2p, \
             tc.tile_pool(name="prevp", bufs=3) as prevp, \
             tc.tile_pool(name="outp", bufs=2) as outp, \
             tc.tile_pool(name="psA", bufs=4, space="PSUM") as psA, \
             tc.tile_pool(name="psB", bufs=2, space="PSUM") as psB, \
             tc.tile_pool(name="psC", bufs=2, space="PSUM") as psC:

            # ---- constant loads ----
            bias_sb = const.tile([128, NBIAS], f32, tag="bias")
            nc.sync.dma_start(bias_sb[:], biases[:])
            cfl_sb = const.tile([128, KC * R], f16, tag="cfl")
            for k in range(KC):
                nc.sync.dma_start(cfl_sb[:, k * R:(k + 1) * R], cflatT[k])
            wp0_sb = const.tile([128, MG * 128], f16, tag="wp0")
            nc.sync.dma_start(wp0_sb[:], wp0[:])
            wp1_sb = const.tile([2, MG * 128], f16, tag="wp1")
            nc.sync.dma_start(wp1_sb[:], wp1[:])
            wfco_sb = const.tile([128, KH * D], f16, tag="wfco")
            nc.sync.dma_start(wfco_sb[:], wfco[:])

            def bias_ap(col):
                return bias_sb[:, col:col + 1]

            # ---- h init: t0T = tanh(wini @ cflatT + binit) ----
            h1T = state.tile([128, KH * R], f16, tag="h1")
            h2T = state.tile([128, KH * R], f16, tag="h2")
            for m in range(MI):
                wchunk = stream.tile([128, KC * 128], f16, tag="stream")
                nc.sync.dma_start(wchunk[:], wini[m])
                ps = psA.tile([128, R], f32, tag="rz")
                for k in range(KC):
                    nc.tensor.matmul(ps[:], wchunk[:, k * 128:(k + 1) * 128],
                                     cfl_sb[:, k * R:(k + 1) * R],
                                     start=(k == 0), stop=(k == KC - 1))
                dst = h1T if m < NJ else h2T
                j = m % NJ
                nc.scalar.activation(dst[:, j * R:(j + 1) * R], ps[:], Tanh,
                                     bias=bias_ap(B_INIT + m))

            # ---- cached = Wc @ cflatT + bih1 ----
            # lives in the gi1 streaming pool: dead once the step loop starts
            cached_sb = gi1p.tile([128, MG * R], f16, tag="gi1")
            for m in range(MG):
                wchunk = stream.tile([128, KC * 128], f16, tag="stream")
                nc.sync.dma_start(wchunk[:], wc[m])
                ps = psA.tile([128, R], f32, tag="rz")
                for k in range(KC):
                    nc.tensor.matmul(ps[:], wchunk[:, k * 128:(k + 1) * 128],
                                     cfl_sb[:, k * R:(k + 1) * R],
                                     start=(k == 0), stop=(k == KC - 1))
                nc.scalar.activation(cached_sb[:, m * R:(m + 1) * R], ps[:], Ident,
                                     bias=bias_ap(B_IH1 + m))

            # ---- gi1[s] = Wp @ prevT + cached, batched over (s, r) ----
            for ch in range(NCH):
                pv0c = stream.tile([128, 2 * R], f16, tag="stream")
                nc.sync.dma_start(pv0c[:], prevT0[:, ch * 2 * R:(ch + 1) * 2 * R])
                pv1c = prevp.tile([2, 2 * R], f16, tag="pv1")
                nc.sync.dma_start(pv1c[:], prevT1[:, ch * 2 * R:(ch + 1) * 2 * R])
                for m in range(MG):
                    ps = psA.tile([128, 2 * R], f32, tag="rz")
                    nc.tensor.matmul(ps[:], wp0_sb[:, m * 128:(m + 1) * 128],
                                     pv0c[:], start=True, stop=False)
                    nc.tensor.matmul(ps[:], wp1_sb[:, m * 128:(m + 1) * 128],
                                     pv1c[:], start=False, stop=True)
                    giw = giwp.tile([128, 2 * R], f16, tag="giw")
                    for half in range(2):
                        nc.vector.tensor_add(giw[:, half * R:(half + 1) * R],
                                             ps[:, half * R:(half + 1) * R],
                                             cached_sb[:, m * R:(m + 1) * R])
                    nc.gpsimd.dma_start(
                        gi1d[ch, :, m * 2 * R:(m + 1) * 2 * R], giw[:])

            # ---- resident big weights ----
            w1h_sb = const.tile([128, MG * KH * 128], f16, tag="w1h")
            for m in range(MG):
                nc.sync.dma_start(w1h_sb[:, m * 1024:(m + 1) * 1024], w1h[m])
            w2i_sb = const.tile([128, MG * KH * 128], f16, tag="w2i")
            for m in range(MG):
                nc.sync.dma_start(w2i_sb[:, m * 1024:(m + 1) * 1024], w2i[m])

            def w1h_t(m, k):
                return w1h_sb[:, m * 1024 + k * 128: m * 1024 + (k + 1) * 128]

            def w2i_t(m, k):
                return w2i_sb[:, m * 1024 + k * 128: m * 1024 + (k + 1) * 128]

            def hslice(t, j):
                return t[:, j * R:(j + 1) * R]

            def fco_step(h2T_cur, s):
                for mo, msz, bc in [(0, 128, B_FCO), (128, 2, B_FCO + 1)]:
                    ps = psA.tile([128, R], f32, tag="rz")
                    for k in range(KH):
                        nc.tensor.matmul(ps[0:msz, :],
                                         wfco_sb[:, k * D + mo: k * D + mo + msz],
                                         hslice(h2T_cur, k),
                                         start=(k == 0), stop=(k == KH - 1))
                    ysb = outp.tile([128, R], f32, tag="y")
                    nc.scalar.activation(ysb[0:msz, :], ps[0:msz, :], Ident,
                                         bias=bias_sb[0:msz, bc:bc + 1])
                    nc.sync.dma_start(yT[s, mo:mo + msz, :], ysb[0:msz, :])

            h2T_done = []  # (h2T tile, step) pending fco

            for s in range(S):
                gi1_t = gi1p.tile([128, MG * R], f16, tag="gi1")
                ch, half = s // 2, s % 2
                src = gi1d[ch].rearrange("p (m x) -> p m x", m=MG)[
                    :, :, half * R:(half + 1) * R]
                nc.sync.dma_start(
                    gi1_t[:].rearrange("p (m r) -> p m r", m=MG), src)

                # ---------- GRU1 ----------
                r1 = gates.tile([128, NJ * R], f16, tag="rg")
                z1 = gates.tile([128, NJ * R], f16, tag="zg")
                n1 = gates.tile([128, NJ * R], f16, tag="ng")
                for m in range(MRZ):
                    ps = psA.tile([128, R], f32, tag="rz")
                    for k in range(KH):
                        nc.tensor.matmul(ps[:], w1h_t(m, k), hslice(h1T, k),
                                         start=(k == 0), stop=(k == KH - 1))
                    nc.vector.tensor_add(ps[:], ps[:], gi1_t[:, m * R:(m + 1) * R])
                    dst = r1 if m < NJ else z1
                    nc.scalar.activation(hslice(dst, m % NJ), ps[:], Sig,
                                         bias=bias_ap(B_RZ1 + m))
                h1T_new = state.tile([128, KH * R], f16, tag="h1")
                for j in range(NJ):
                    m = MRZ + j
                    psh = psC.tile([128, R], f32, tag="ghn")
                    for k in range(KH):
                        nc.tensor.matmul(psh[:], w1h_t(m, k), hslice(h1T, k),
                                         start=(k == 0), stop=(k == KH - 1))
                    ghb = tmp.tile([128, R], f16, tag="ghb")
                    nc.scalar.activation(ghb[:], psh[:], Ident, bias=bias_ap(B_N1H + j))
                    tt = tmp.tile([128, R], f16, tag="tt")
                    nc.vector.tensor_mul(tt[:], hslice(r1, j), ghb[:])
                    narg = tmp.tile([128, R], f16, tag="narg")
                    nc.vector.tensor_add(narg[:], tt[:], gi1_t[:, m * R:(m + 1) * R])
                    nc.scalar.activation(hslice(n1, j), narg[:], Tanh, bias=0.0)
                    d = tmp.tile([128, R], f16, tag="d")
                    nc.vector.tensor_sub(d[:], hslice(h1T, j), hslice(n1, j))
                    nc.vector.tensor_mul(d[:], hslice(z1, j), d[:])
                    nc.vector.tensor_add(hslice(h1T_new, j), hslice(n1, j), d[:])

                # fco for previous step (PE filler while GRU1 gates finish)
                if h2T_done:
                    fco_step(*h2T_done.pop())

                # ---------- GRU2 ----------
                r2 = gates.tile([128, NJ * R], f16, tag="rg")
                z2 = gates.tile([128, NJ * R], f16, tag="zg")
                n2 = gates.tile([128, NJ * R], f16, tag="ng")
                # n-gate gh part first: independent of h1T_new
                ghb2s = []
                for j in range(NJ):
                    m = MRZ + j
                    wch = stream.tile([128, KH * 128], f16, tag="stream")
                    nc.sync.dma_start(wch[:], w2h[m])
                    psh = psC.tile([128, R], f32, tag="ghn")
                    for k in range(KH):
                        nc.tensor.matmul(psh[:], wch[:, k * 128:(k + 1) * 128],
                                         hslice(h2T, k),
                                         start=(k == 0), stop=(k == KH - 1))
                    ghb2 = ghb2p.tile([128, R], f16, tag="ghb2")
                    nc.scalar.activation(ghb2[:], psh[:], Ident, bias=bias_ap(B_N2H + j))
                    ghb2s.append(ghb2)
                for m in range(MRZ):
                    wch = stream.tile([128, KH * 128], f16, tag="stream")
                    nc.sync.dma_start(wch[:], w2h[m])
                    ps = psA.tile([128, R], f32, tag="rz")
                    for k in range(KH):
                        nc.tensor.matmul(ps[:], wch[:, k * 128:(k + 1) * 128],
                                         hslice(h2T, k),
                                         start=(k == 0), stop=False)
                    for k in range(KH):
                        nc.tensor.matmul(ps[:], w2i_t(m, k), hslice(h1T_new, k),
                                         start=False, stop=(k == KH - 1))
                    dst = r2 if m < NJ else z2
                    nc.scalar.activation(hslice(dst, m % NJ), ps[:], Sig,
                                         bias=bias_ap(B_RZ2 + m))
                h2T_new = state.tile([128, KH * R], f16, tag="h2")
                for j in range(NJ):
                    m = MRZ + j
                    psg = psB.tile([128, R], f32, tag="gin")
                    for k in range(KH):
                        nc.tensor.matmul(psg[:], w2i_t(m, k), hslice(h1T_new, k),
                                         start=(k == 0), stop=(k == KH - 1))
                    tt = tmp.tile([128, R], f16, tag="tt")
                    nc.vector.tensor_mul(tt[:], hslice(r2, j), ghb2s[j][:])
                    nc.vector.tensor_add(psg[:], psg[:], tt[:])
                    nc.scalar.activation(hslice(n2, j), psg[:], Tanh,
                                         bias=bias_ap(B_N2I + j))
                    d = tmp.tile([128, R], f16, tag="d")
                    nc.vector.tensor_sub(d[:], hslice(h2T, j), hslice(n2, j))
                    nc.vector.tensor_mul(d[:], hslice(z2, j), d[:])
                    nc.vector.tensor_add(hslice(h2T_new, j), hslice(n2, j), d[:])

                h1T, h2T = h1T_new, h2T_new
                h2T_done.append((h2T, s))

            fco_step(*h2T_done.pop())

    nc.compile()
    return nc


def prep_inputs(c, target, fc_init_w, fc_init_b, g1_wih, g1_whh, g1_bih, g1_bhh,
                g2_wih, g2_whh, g2_bih, g2_bhh, fco_w, fco_b):
    """Host-side shard/layout prep. Returns per-core input maps."""
    f16 = np.float16
    T = E * S
    c = np.asarray(c, np.float32)
    target = np.asarray(target, np.float32)

    # weights: stationary-tile layouts (shared across cores)
    w1h_a = _wtiles(np.asarray(g1_whh, np.float32).T, MG, KH)
    w2i_a = _wtiles(np.asarray(g2_wih, np.float32).T, MG, KH)
    w2h_a = _wtiles(np.asarray(g2_whh, np.float32).T, MG, KH)
    wc_a = _wtiles(np.asarray(g1_wih, np.float32)[:, :C].T, MG, KC)
    wini_a = _wtiles(np.asarray(fc_init_w, np.float32).T, MI, KC)
    wp_t = np.asarray(g1_wih, np.float32)[:, C:].T      # [130, 3072]
    wp0_a = np.ascontiguousarray(wp_t[:128]).astype(f16)   # [128, 24*128]
    wp1_a = np.ascontiguousarray(wp_t[128:]).astype(f16)   # [2, 24*128]
    wfco_a = np.ascontiguousarray(
        np.asarray(fco_w, np.float32).T.reshape(KH, 128, D)
        .transpose(1, 0, 2).reshape(128, KH * D)).astype(f16)

    bias = np.zeros((128, NBIAS), np.float32)
    bias[:, B_INIT:B_INIT + MI] = _bias_cols(np.asarray(fc_init_b, np.float32), MI)
    bhh1 = np.asarray(g1_bhh, np.float32)
    bih1 = np.asarray(g1_bih, np.float32)
    bhh2 = np.asarray(g2_bhh, np.float32)
    bih2 = np.asarray(g2_bih, np.float32)
    bias[:, B_RZ1:B_RZ1 + 16] = _bias_cols(bhh1[:2 * H], 16)
    bias[:, B_N1H:B_N1H + 8] = _bias_cols(bhh1[2 * H:], 8)
    bias[:, B_IH1:B_IH1 + 24] = _bias_cols(bih1, 24)
    bias[:, B_RZ2:B_RZ2 + 16] = _bias_cols(bih2[:2 * H] + bhh2[:2 * H], 16)
    bias[:, B_N2H:B_N2H + 8] = _bias_cols(bhh2[2 * H:], 8)
    bias[:, B_N2I:B_N2I + 8] = _bias_cols(bih2[2 * H:], 8)
    fco_b = np.asarray(fco_b, np.float32)
    bias[:, B_FCO] = fco_b[:128]
    bias[0:2, B_FCO + 1] = fco_b[128:130]

    prev_full = np.concatenate(
        [np.zeros((B, 1, D), np.float32), target[:, :T - 1]], axis=1)  # [B,T,D]

    in_maps = []
    for core in range(NCORES):
        e0 = core * EPC
        cf = c[e0:e0 + EPC].reshape(R, C)                  # [256, 512]
        cfT = np.ascontiguousarray(cf.T.reshape(KC, 128, R)).astype(f16)
        pv = prev_full[:, e0 * S:(e0 + EPC) * S]           # [B, 32, D]
        pv = pv.reshape(B, EPC, S, D).transpose(2, 1, 0, 3).reshape(S, R, D)
        pvT = np.ascontiguousarray(pv.transpose(2, 0, 1).reshape(D, S * R))  # [D,(s,r)]
        in_maps.append({
            "cflatT": cfT,
            "prevT0": np.ascontiguousarray(pvT[:128]).astype(f16),
            "prevT1": np.ascontiguousarray(pvT[128:130]).astype(f16),
            "w1h": w1h_a, "w2i": w2i_a, "w2h": w2h_a,
            "wp0": wp0_a, "wp1": wp1_a, "wc": wc_a, "wini": wini_a,
            "wfco": wfco_a, "biases": bias,
        })
    return in_maps


def assemble_output(results):
    """Per-core yT [S, 132, R] f32 -> full [B, T, D] f32."""
    T = E * S
    out = np.empty((B, T, D), np.float32)
    for core in range(NCORES):
        yt = results[core]["yT"]            # [S, 132, R]
        for ei in range(EPC):
            e = core * EPC + ei
            blk = yt[:, :D, ei * 128:(ei + 1) * 128]   # [S, D, 128]
            out[:, e * S:(e + 1) * S, :] = blk.transpose(2, 0, 1)
    return out


def kernel(c, target, length, batch_size, fc_init_w, fc_init_b,
           g1_wih, g1_whh, g1_bih, g1_bhh,
           g2_wih, g2_whh, g2_bih, g2_bhh, fco_w, fco_b):
    from concourse.bass_utils import run_bass_kernel_spmd

    if "nc" not in _cache:
        _cache["nc"] = build_program()
    nc = _cache["nc"]
    in_maps = prep_inputs(c, target, fc_init_w, fc_init_b,
                          g1_wih, g1_whh, g1_bih, g1_bhh,
                          g2_wih, g2_whh, g2_bih, g2_bhh, fco_w, fco_b)
    res = run_bass_kernel_spmd(nc, in_maps, list(range(NCORES)))
    return assemble_output(res.results)


# revision 14
# speedup vs baseline: 1.0280x; 1.0038x over previous
"""Trainium2 Bass kernel for nn_BottomLevelDecoderRNN.

2-layer GRU decoder, H=1024, over S=16 steps for E*B = 16*128 = 2048
independent sequences. Data-parallel over 8 NeuronCores: each core owns
2 conductor embeddings (256 rows).

Dataflow (per core, everything transposed as [feature, row]):
  init:    h-init: t0T = tanh(fc_init_w @ cflatT + b)  -> h1T, h2T [H, 256]
           cached = Wc @ cflatT + bih1                 (Wc = g1_wih[:, :C])
           gi1[s] = Wp @ prevT[s] + cached  (all s, batched N=512) -> DRAM
  step s:  g1T = whh1 @ h1T  (+ gi1[s] streamed back)  -> GRU1 gates -> h1T'
           g2T = wih2 @ h1T' + whh2 @ h2T              -> GRU2 gates -> h2T'
           yT  = fco_w @ h2T' + fco_b                  -> out
Matmuls run with fp16 operands (weights pre-transposed on host into PE
stationary-tile layout), fp32 PSUM accumulation. whh2 streams from HBM
per step (SBUF cannot hold all three 3072x1024 matrices + state).
"""
import numpy as np

E, B, C, H, D = 16, 128, 512, 1024, 130
S = 16
NCORES = 8
EPC = E // NCORES        # 2 embeddings per core
R = EPC * B              # 256 rows per core
KH = H // 128            # 8 h k-tiles
MG = 3 * H // 128        # 24 gate m-tiles
MRZ = 2 * H // 128       # 16 rz m-tiles
NJ = H // 128            # 8 n/h tiles
KC = C // 128            # 4 c k-tiles
MI = 2 * H // 128        # 16 init m-tiles
NCH = S // 2             # 8 init gi1 chunks (2 steps each, N=512)

# bias tile column layout ([128, NBIAS] fp32)
B_INIT = 0      # 16: fc_init_b
B_RZ1 = 16      # 16: bhh1[:2H]
B_N1H = 32      # 8:  bhh1[2H:]
B_IH1 = 40      # 24: bih1
B_RZ2 = 64      # 16: bih2[:2H]+bhh2[:2H]
B_N2H = 80      # 8:  bhh2[2H:]
B_N2I = 88      # 8:  bih2[2H:]
B_FCO = 96      # 2:  fco_b
NBIAS = 98

_cache = {}


def _wtiles(w_t, nm, nk):
    """[Kfull, Mfull] (already transposed: w_t = W.T) -> [nm, 128, nk*128]
    stationary-tile chunks: chunk[m][p, k*128+c] = w_t[128k+p, 128m+c]."""
    Kf, Mf = w_t.shape
    assert Kf == nk * 128 and Mf == nm * 128
    return np.ascontiguousarray(
        w_t.reshape(nk, 128, nm, 128).transpose(2, 1, 0, 3).reshape(nm, 128, nk * 128)
    ).astype(np.float16)


def _bias_cols(vec, n):
    """[n*128] -> [128, n] (col j = vec[128j:128j+128])"""
    return np.ascontiguousarray(vec.reshape(n, 128).T).astype(np.float32)


def build_program():
    import concourse.tile as tile
    from concourse import bacc, mybir

    f32, f16 = mybir.dt.float32, mybir.dt.float16
    Sig = mybir.ActivationFunctionType.Sigmoid
    Tanh = mybir.ActivationFunctionType.Tanh
    Ident = mybir.ActivationFunctionType.Identity

    nc = bacc.Bacc("TRN2", target_bir_lowering=False, debug=False,
                   enable_asserts=False, num_devices=NCORES)

    def din(name, shape, dt=f16):
        return nc.dram_tensor(name, shape, dt, kind="ExternalInput").ap()

    cflatT = din("cflatT", [KC, 128, R])
    prevT0 = din("prevT0", [128, S * R])     # columns ordered (s, r)
    prevT1 = din("prevT1", [2, S * R])
    w1h = din("w1h", [MG, 128, KH * 128])
    w2i = din("w2i", [MG, 128, KH * 128])
    w2h = din("w2h", [MG, 128, KH * 128])
    wp0 = din("wp0", [128, MG * 128])
    wp1 = din("wp1", [2, MG * 128])
    wc = din("wc", [MG, 128, KC * 128])
    wini = din("wini", [MI, 128, KC * 128])
    wfco = din("wfco", [128, KH * D])
    biases = din("biases", [128, NBIAS], f32)
    yT = nc.dram_tensor("yT", [S, 132, R], f32, kind="ExternalOutput").ap()
    # internal scratch: per chunk ch (= steps 2ch, 2ch+1), m-block layout
    # [128, m*512 + (half*256 + r)]
    gi1d = nc.dram_tensor("gi1d", [NCH, 128, MG * 2 * R], f16).ap()

    with tile.TileContext(nc) as tc:
        with tc.tile_pool(name="const", bufs=1) as const, \
             tc.tile_pool(name="stream", bufs=8) as stream, \
             tc.tile_pool(name="gi1p", bufs=2) as gi1p, \
             tc.tile_pool(name="state", bufs=2) as state, \
             tc.tile_pool(name="gates", bufs=1) as gates, \
             tc.tile_pool(name="tmp", bufs=3) as tmp, \
             tc.tile_pool(name="giwp", bufs=5) as giwp, \
             tc.tile_pool(name="ghb2p", bufs=9) as ghb2p, \
             tc.tile_pool(name="prevp", bufs=3) as prevp, \
             tc.tile_pool(name="outp", bufs=2) as outp, \
             tc.tile_pool(name="psA", bufs=4, space="PSUM") as psA, \
             tc.tile_pool(name="psB", bufs=2, space="PSUM") as psB, \
             tc.tile_pool(name="psC", bufs=2, space="PSUM") as psC:

            # ---- constant loads ----
            bias_sb = const.tile([128, NBIAS], f32, tag="bias")
            nc.sync.dma_start(bias_sb[:], biases[:])
            cfl_sb = const.tile([128, KC * R], f16, tag="cfl")
            for k in range(KC):
                nc.sync.dma_start(cfl_sb[:, k * R:(k + 1) * R], cflatT[k])
            wp0_sb = const.tile([128, MG * 128], f16, tag="wp0")
            nc.sync.dma_start(wp0_sb[:], wp0[:])
            wp1_sb = const.tile([2, MG * 128], f16, tag="wp1")
            nc.sync.dma_start(wp1_sb[:], wp1[:])
            wfco_sb = const.tile([128, KH * D], f16, tag="wfco")
            nc.sync.dma_start(wfco_sb[:], wfco[:])

            def bias_ap(col):
                return bias_sb[:, col:col + 1]

            # ---- h init: t0T = tanh(wini @ cflatT + binit) ----
            h1T = state.tile([128, KH * R], f16, tag="h1")
            h2T = state.tile([128, KH * R], f16, tag="h2")
            for m in range(MI):
                wchunk = stream.tile([128, KC * 128], f16, tag="stream")
                nc.sync.dma_start(wchunk[:], wini[m])
                ps = psA.tile([128, R], f32, tag="rz")
                for k in range(KC):
                    nc.tensor.matmul(ps[:], wchunk[:, k * 128:(k + 1) * 128],
                                     cfl_sb[:, k * R:(k + 1) * R],
                                     start=(k == 0), stop=(k == KC - 1))
                dst = h1T if m < NJ else h2T
                j = m % NJ
                nc.scalar.activation(dst[:, j * R:(j + 1) * R], ps[:], Tanh,
                                     bias=bias_ap(B_INIT + m))

            # ---- cached = Wc @ cflatT + bih1 ----
            # lives in the gi1 streaming pool: dead once the step loop starts
            cached_sb = gi1p.tile([128, MG * R], f16, tag="gi1")
            for m in range(MG):
                wchunk = stream.tile([128, KC * 128], f16, tag="stream")
                nc.sync.dma_start(wchunk[:], wc[m])
                ps = psA.tile([128, R], f32, tag="rz")
                for k in range(KC):
                    nc.tensor.matmul(ps[:], wchunk[:, k * 128:(k + 1) * 128],
                                     cfl_sb[:, k * R:(k + 1) * R],
                                     start=(k == 0), stop=(k == KC - 1))
                nc.scalar.activation(cached_sb[:, m * R:(m + 1) * R], ps[:], Ident,
                                     bias=bias_ap(B_IH1 + m))

            # ---- gi1[s] = Wp @ prevT + cached, batched over (s, r) ----
            for ch in range(NCH):
                pv0c = stream.tile([128, 2 * R], f16, tag="stream")
                nc.sync.dma_start(pv0c[:], prevT0[:, ch * 2 * R:(ch + 1) * 2 * R])
                pv1c = prevp.tile([2, 2 * R], f16, tag="pv1")
                nc.sync.dma_start(pv1c[:], prevT1[:, ch * 2 * R:(ch + 1) * 2 * R])
                for mp in range(MG // 2):
                    giw = giwp.tile([128, 4 * R], f16, tag="giw")
                    for q in range(2):
                        m = 2 * mp + q
                        ps = psA.tile([128, 2 * R], f32, tag="rz")
                        nc.tensor.matmul(ps[:], wp0_sb[:, m * 128:(m + 1) * 128],
                                         pv0c[:], start=True, stop=False)
                        nc.tensor.matmul(ps[:], wp1_sb[:, m * 128:(m + 1) * 128],
                                         pv1c[:], start=False, stop=True)
                        cslc = cached_sb[:, m * R:(m + 1) * R]
                        nc.vector.tensor_add(
                            giw[:, q * 2 * R:(q + 1) * 2 * R].rearrange(
                                "p (h r) -> p h r", h=2),
                            ps[:].rearrange("p (h r) -> p h r", h=2),
                            cslc.unsqueeze(1).broadcast_to([128, 2, R]))
                    nc.scalar.dma_start(
                        gi1d[ch, :, mp * 4 * R:(mp + 1) * 4 * R], giw[:])

            # ---- resident big weights ----
            w1h_sb = const.tile([128, MG * KH * 128], f16, tag="w1h")
            for m in range(MG):
                nc.sync.dma_start(w1h_sb[:, m * 1024:(m + 1) * 1024], w1h[m])
            w2i_sb = const.tile([128, MG * KH * 128], f16, tag="w2i")
            for m in range(MG):
                nc.sync.dma_start(w2i_sb[:, m * 1024:(m + 1) * 1024], w2i[m])

            def w1h_t(m, k):
                return w1h_sb[:, m * 1024 + k * 128: m * 1024 + (k + 1) * 128]

            def w2i_t(m, k):
                return w2i_sb[:, m * 1024 + k * 128: m * 1024 + (k + 1) * 128]

            def hslice(t, j):
                return t[:, j * R:(j + 1) * R]

            def fco_step(h2T_cur, s):
                for mo, msz, bc in [(0, 128, B_FCO), (128, 2, B_FCO + 1)]:
                    ps = psA.tile([128, R], f32, tag="rz")
                    for k in range(KH):
                        nc.tensor.matmul(ps[0:msz, :],
                                         wfco_sb[:, k * D + mo: k * D + mo + msz],
                                         hslice(h2T_cur, k),
                                         start=(k == 0), stop=(k == KH - 1))
                    ysb = outp.tile([128, R], f32, tag="y")
                    nc.scalar.activation(ysb[0:msz, :], ps[0:msz, :], Ident,
                                         bias=bias_sb[0:msz, bc:bc + 1])
                    nc.sync.dma_start(yT[s, mo:mo + msz, :], ysb[0:msz, :])

            h2T_done = []  # (h2T tile, step) pending fco

            for s in range(S):
                gi1_t = gi1p.tile([128, MG * R], f16, tag="gi1")
                ch, half = s // 2, s % 2
                src = gi1d[ch].rearrange("p (m x) -> p m x", m=MG)[
                    :, :, half * R:(half + 1) * R]
                nc.sync.dma_start(
                    gi1_t[:].rearrange("p (m r) -> p m r", m=MG), src)

                # ---------- GRU1 ----------
                r1 = gates.tile([128, NJ * R], f16, tag="rg")
                z1 = gates.tile([128, NJ * R], f16, tag="zg")
                n1 = gates.tile([128, NJ * R], f16, tag="ng")
                for m in range(MRZ):
                    ps = psA.tile([128, R], f32, tag="rz")
                    for k in range(KH):
                        nc.tensor.matmul(ps[:], w1h_t(m, k), hslice(h1T, k),
                                         start=(k == 0), stop=(k == KH - 1))
                    nc.vector.tensor_add(ps[:], ps[:], gi1_t[:, m * R:(m + 1) * R])
                    dst = r1 if m < NJ else z1
                    nc.scalar.activation(hslice(dst, m % NJ), ps[:], Sig,
                                         bias=bias_ap(B_RZ1 + m))
                h1T_new = state.tile([128, KH * R], f16, tag="h1")
                for j in range(NJ):
                    m = MRZ + j
                    psh = psC.tile([128, R], f32, tag="ghn")
                    for k in range(KH):
                        nc.tensor.matmul(psh[:], w1h_t(m, k), hslice(h1T, k),
                                         start=(k == 0), stop=(k == KH - 1))
                    ghb = tmp.tile([128, R], f16, tag="ghb")
                    nc.scalar.activation(ghb[:], psh[:], Ident, bias=bias_ap(B_N1H + j))
                    tt = tmp.tile([128, R], f16, tag="tt")
                    nc.vector.tensor_mul(tt[:], hslice(r1, j), ghb[:])
                    narg = tmp.tile([128, R], f16, tag="narg")
                    nc.vector.tensor_add(narg[:], tt[:], gi1_t[:, m * R:(m + 1) * R])
                    nc.scalar.activation(hslice(n1, j), narg[:], Tanh, bias=0.0)
                    d = tmp.tile([128, R], f16, tag="d")
                    nc.vector.tensor_sub(d[:], hslice(h1T, j), hslice(n1, j))
                    nc.vector.tensor_mul(d[:], hslice(z1, j), d[:])
                    nc.vector.tensor_add(hslice(h1T_new, j), hslice(n1, j), d[:])

                # fco for previous step (PE filler while GRU1 gates finish)
                if h2T_done:
                    fco_step(*h2T_done.pop())

                # ---------- GRU2 ----------
                r2 = gates.tile([128, NJ * R], f16, tag="rg")
                z2 = gates.tile([128, NJ * R], f16, tag="zg")
                n2 = gates.tile([128, NJ * R], f16, tag="ng")
                # n-gate gh part first: independent of h1T_new
                ghb2s = []
                for j in range(NJ):
                    m = MRZ + j
                    wch = stream.tile([128, KH * 128], f16, tag="stream")
                    nc.sync.dma_start(wch[:], w2h[m])
                    psh = psC.tile([128, R], f32, tag="ghn")
                    for k in range(KH):
                        nc.tensor.matmul(psh[:], wch[:, k * 128:(k + 1) * 128],
                                         hslice(h2T, k),
                                         start=(k == 0), stop=(k == KH - 1))
                    ghb2 = ghb2p.tile([128, R], f16, tag="ghb2")
                    nc.scalar.activation(ghb2[:], psh[:], Ident, bias=bias_ap(B_N2H + j))
                    ghb2s.append(ghb2)
                for m in range(MRZ):
                    wch = stream.tile([128, KH * 128], f16, tag="stream")
                    nc.sync.dma_start(wch[:], w2h[m])
                    ps = psA.tile([128, R], f32, tag="rz")
                    for k in range(KH):
                        nc.tensor.matmul(ps[:], wch[:, k * 128:(k + 1) * 128],
                                         hslice(h2T, k),
                                         start=(k == 0), stop=False)
                    for k in range(KH):
                        nc.tensor.matmul(ps[:], w2i_t(m, k), hslice(h1T_new, k),
                                         start=False, stop=(k == KH - 1))
                    dst = r2 if m < NJ else z2
                    nc.scalar.activation(hslice(dst, m % NJ), ps[:], Sig,
                                         bias=bias_ap(B_RZ2 + m))
                h2T_new = state.tile([128, KH * R], f16, tag="h2")
                for j in range(NJ):
                    m = MRZ + j
                    psg = psB.tile([128, R], f32, tag="gin")
                    for k in range(KH):
                        nc.tensor.matmul(psg[:], w2i_t(m, k), hslice(h1T_new, k),
                                         start=(k == 0), stop=(k == KH - 1))
                    tt = tmp.tile([128, R], f16, tag="tt")
                    nc.vector.tensor_mul(tt[:], hslice(r2, j), ghb2s[j][:])
                    nc.vector.tensor_add(psg[:], psg[:], tt[:])
                    nc.scalar.activation(hslice(n2, j), psg[:], Tanh,
                                         bias=bias_ap(B_N2I + j))
                    d = tmp.tile([128, R], f16, tag="d")
                    nc.vector.tensor_sub(d[:], hslice(h2T, j), hslice(n2, j))
                    nc.vector.tensor_mul(d[:], hslice(z2, j), d[:])
                    nc.vector.tensor_add(hslice(h2T_new, j), hslice(n2, j), d[:])

                h1T, h2T = h1T_new, h2T_new
                h2T_done.append((h2T, s))

            fco_step(*h2T_done.pop())

    nc.compile()
    return nc


def prep_inputs(c, target, fc_init_w, fc_init_b, g1_wih, g1_whh, g1_bih, g1_bhh,
                g2_wih, g2_whh, g2_bih, g2_bhh, fco_w, fco_b):
    """Host-side shard/layout prep. Returns per-core input maps."""
    f16 = np.float16
    T = E * S
    c = np.asarray(c, np.float32)
    target = np.asarray(target, np.float32)

    # weights: stationary-tile layouts (shared across cores)
    w1h_a = _wtiles(np.asarray(g1_whh, np.float32).T, MG, KH)
    w2i_a = _wtiles(np.asarray(g2_wih, np.float32).T, MG, KH)
    w2h_a = _wtiles(np.asarray(g2_whh, np.float32).T, MG, KH)
    wc_a = _wtiles(np.asarray(g1_wih, np.float32)[:, :C].T, MG, KC)
    wini_a = _wtiles(np.asarray(fc_init_w, np.float32).T, MI, KC)
    wp_t = np.asarray(g1_wih, np.float32)[:, C:].T      # [130, 3072]
    wp0_a = np.ascontiguousarray(wp_t[:128]).astype(f16)   # [128, 24*128]
    wp1_a = np.ascontiguousarray(wp_t[128:]).astype(f16)   # [2, 24*128]
    wfco_a = np.ascontiguousarray(
        np.asarray(fco_w, np.float32).T.reshape(KH, 128, D)
        .transpose(1, 0, 2).reshape(128, KH * D)).astype(f16)

    bias = np.zeros((128, NBIAS), np.float32)
    bias[:, B_INIT:B_INIT + MI] = _bias_cols(np.asarray(fc_init_b, np.float32), MI)
    bhh1 = np.asarray(g1_bhh, np.float32)
    bih1 = np.asarray(g1_bih, np.float32)
    bhh2 = np.asarray(g2_bhh, np.float32)
    bih2 = np.asarray(g2_bih, np.float32)
    bias[:, B_RZ1:B_RZ1 + 16] = _bias_cols(bhh1[:2 * H], 16)
    bias[:, B_N1H:B_N1H + 8] = _bias_cols(bhh1[2 * H:], 8)
    bias[:, B_IH1:B_IH1 + 24] = _bias_cols(bih1, 24)
    bias[:, B_RZ2:B_RZ2 + 16] = _bias_cols(bih2[:2 * H] + bhh2[:2 * H], 16)
    bias[:, B_N2H:B_N2H + 8] = _bias_cols(bhh2[2 * H:], 8)
    bias[:, B_N2I:B_N2I + 8] = _bias_cols(bih2[2 * H:], 8)
    fco_b = np.asarray(fco_b, np.float32)
    bias[:, B_FCO] = fco_b[:128]
    bias[0:2, B_FCO + 1] = fco_b[128:130]

    prev_full = np.concatenate(
        [np.zeros((B, 1, D), np.float32), target[:, :T - 1]], axis=1)  # [B,T,D]

    in_maps = []
    for core in range(NCORES):
        e0 = core * EPC
        cf = c[e0:e0 + EPC].reshape(R, C)                  # [256, 512]
        cfT = np.ascontiguousarray(cf.T.reshape(KC, 128, R)).astype(f16)
        pv = prev_full[:, e0 * S:(e0 + EPC) * S]           # [B, 32, D]
        pv = pv.reshape(B, EPC, S, D).transpose(2, 1, 0, 3).reshape(S, R, D)
        pvT = np.ascontiguousarray(pv.transpose(2, 0, 1).reshape(D, S * R))  # [D,(s,r)]
        in_maps.append({
            "cflatT": cfT,
            "prevT0": np.ascontiguousarray(pvT[:128]).astype(f16),
            "prevT1": np.ascontiguousarray(pvT[128:130]).astype(f16),
            "w1h": w1h_a, "w2i": w2i_a, "w2h": w2h_a,
            "wp0": wp0_a, "wp1": wp1_a, "wc": wc_a, "wini": wini_a,
            "wfco": wfco_a, "biases": bias,
        })
    return in_maps


def assemble_output(results):
    """Per-core yT [S, 132, R] f32 -> full [B, T, D] f32."""
    T = E * S
    out = np.empty((B, T, D), np.float32)
    for core in range(NCORES):
        yt = results[core]["yT"]            # [S, 132, R]
        for ei in range(EPC):
            e = core * EPC + ei
            blk = yt[:, :D, ei * 128:(ei + 1) * 128]   # [S, D, 128]
            out[:, e * S:(e + 1) * S, :] = blk.transpose(2, 0, 1)
    return out


def kernel(c, target, length, batch_size, fc_init_w, fc_init_b,
           g1_wih, g1_whh, g1_bih, g1_bhh,
           g2_wih, g2_whh, g2_bih, g2_bhh, fco_w, fco_b):
    from concourse.bass_utils import run_bass_kernel_spmd

    if "nc" not in _cache:
        _cache["nc"] = build_program()
    nc = _cache["nc"]
    in_maps = prep_inputs(c, target, fc_init_w, fc_init_b,
                          g1_wih, g1_whh, g1_bih, g1_bhh,
                          g2_wih, g2_whh, g2_bih, g2_bhh, fco_w, fco_b)
    res = run_bass_kernel_spmd(nc, in_maps, list(range(NCORES)))
    return assemble_output(res.results)
